# revision 36
# baseline (speedup 1.0000x reference)
# GATConv Trainium kernel: host prep + Bass program builder (parameterized).
# One-hot scatter/gather matrices are built ON-CHIP (DVE is_equal + PE
# transpose) from a 1-byte-per-edge dst-local index stream; gather tables are
# written p-major so HWDGE writes are page-local.
import numpy as np
import ml_dtypes
import concourse.bass as bass
import concourse.bacc as bacc
import concourse.mybir as mybir
import concourse.tile as tile
from concourse._compat import exact_div

F32 = mybir.dt.float32
BF16 = mybir.dt.bfloat16
I16 = mybir.dt.int16
FP8 = mybir.dt.float8e4

ALPHA = 0.2
H, D = 8, 32
HD = H * D            # 256
IN = 256
FT_W = 384            # padded ft row (bf16) -> 768B stride; cols 0:256 ft, 256:264 el bf16
ROW = HD + 8          # 264 gathered cols per edge


def _ceil(a, b):
    return -(-a // b)


class Plan:
    """Host-side uniform schedule shared by all cores.

    Chunk-major, one dst tile per call (ST=1): for q in chunks, for t in
    tiles, one call of ceil(max-over-cores-count/128) 128-slot blocks.
    Every block belongs to exactly one tile, so block == segment: one erx +
    one agg matmul each, with one-hots generated on-chip from dstl bytes.
    Per-tile results accumulate across chunk passes in an SBUF accumulator."""

    def __init__(self, N, E, src, dst, n_cores, tiles_per_core,
                 chunk=32768, wmax=8):
        self.N, self.E, self.C = N, E, n_cores
        self.NT = tiles_per_core              # dst tiles per core
        self.ND = tiles_per_core * 128        # dsts per core
        NNfull = self.ND * n_cores
        assert NNfull >= N
        self.chunk = chunk
        self.WMAX = wmax

        order = np.argsort(dst, kind="stable")
        src_s, dst_s = src[order], dst[order]
        core_of = dst_s // self.ND
        tile_of = (dst_s % self.ND) // 128

        # per-core compacted node table: [own dsts (ND rows, tile order) |
        # other distinct srcs sorted]; comp index addresses the ft table.
        self.node_order = []                  # per core: original node id per row
        comp_of = np.empty(E, dtype=np.int64)  # per (sorted) edge: comp idx of src
        used = 0
        for c in range(n_cores):
            sel = core_of == c
            srcs = src_s[sel]
            own_lo, own_hi = c * self.ND, (c + 1) * self.ND
            others = np.unique(srcs)
            others = others[(others < own_lo) | (others >= own_hi)]
            order_c = np.concatenate([np.arange(own_lo, own_hi), others])
            self.node_order.append(order_c)
            used = max(used, len(order_c))
            # comp idx: own -> src-own_lo ; other -> ND + rank in others
            ci = np.where((srcs >= own_lo) & (srcs < own_hi),
                          srcs - own_lo,
                          self.ND + np.searchsorted(others, srcs))
            comp_of[sel] = ci
        self.NN = _ceil(used, 2048) * 2048    # pad to fc-group multiple
        self.NQ = _ceil(self.NN, chunk)
        # chunk 0 takes the slack so the serial FC prologue (edge pipeline
        # can't start before chunk-0's table + er are complete) is shortest.
        c0 = self.NN - (self.NQ - 1) * chunk
        assert c0 > 0 and c0 % 2048 == 0
        self.bounds = [0] + [c0 + i * chunk for i in range(self.NQ)]
        q_of = np.searchsorted(self.bounds, comp_of, side="right") - 1

        cnt = np.zeros((n_cores, self.NT, self.NQ), dtype=np.int64)
        np.add.at(cnt, (core_of, tile_of, q_of), 1)
        mx = cnt.max(axis=0)                  # [NT, NQ] slots per group
        mx[:, 0] = np.maximum(mx[:, 0], 1)    # every tile appears in chunk 0
        self.maxcnt = mx

        # per-(c,t,q) edge lists (comp src idx, global dst)
        self.edges = [[[None] * self.NQ for _ in range(self.NT)] for _ in range(n_cores)]
        key = ((core_of * self.NT + tile_of) * self.NQ + q_of)
        order2 = np.argsort(key, kind="stable")
        ks = key[order2]
        bounds = np.searchsorted(ks, np.arange(n_cores * self.NT * self.NQ + 1))
        for c in range(n_cores):
            for t in range(self.NT):
                for q in range(self.NQ):
                    k = (c * self.NT + t) * self.NQ + q
                    sel = order2[bounds[k]:bounds[k + 1]]
                    self.edges[c][t][q] = (comp_of[sel], dst_s[sel])

        # calls: chunk-major, tile-minor; every block serves one tile.
        self.calls = []         # (q, t, base_slot, nblocks)
        self.windows = []       # (q, t, wn, slot_pos)
        self.nb = {}            # (q, t) -> blocks (= agg matmuls) of the call
        self.tot_segs = [0] * self.NT
        pos = 0
        for q in range(self.NQ):
            for t in range(self.NT):
                L = int(mx[t, q])
                if L == 0:
                    continue
                nb = _ceil(L, 128)
                self.calls.append((q, t, pos, nb))
                self.nb[(q, t)] = nb
                self.tot_segs[t] += nb
                for w0 in range(0, nb, wmax):
                    wn = min(wmax, nb - w0)
                    self.windows.append((q, t, wn, pos + w0 * 128))
                pos += nb * 128
        self.NBtot = pos

    def build_streams(self, c):
        NB = self.NBtot
        rows_q = [self.bounds[q + 1] - self.bounds[q] for q in range(self.NQ)]
        idx_ft = np.zeros(NB, dtype=np.int64)
        dstl = np.full(NB, 255.0, dtype=np.float32)
        for q, t, base, nbk in self.calls:
            s_arr, d_arr = self.edges[c][t][q]
            n = len(s_arr)
            assert n <= nbk * 128
            r = s_arr - self.bounds[q]          # chunk-local row
            g = rows_q[q] // 128                # rows per partition (p-major)
            idx_ft[base:base + n] = (r % 128) * g + r // 128
            dstl[base:base + n] = (d_arr - (c * self.ND + t * 128)).astype(np.float32)
        assert idx_ft.max() < 32768

        # gather index table: slot s -> partition 16k + s%16, col s//16, x8
        ift = np.zeros((128, NB // 16), dtype=np.int16)
        i = np.arange(NB)
        xs = idx_ft.astype(np.int16)
        for k in range(8):
            ift[16 * k + i % 16, i // 16] = xs
        # dst-local index per slot: block-column layout [128, NB/128] bf16
        dstlT = np.ascontiguousarray(
            dstl.reshape(NB // 128, 128).T).astype(ml_dtypes.bfloat16)
        # streamed transposed one-hot: ohT[d, s] = (dstl[s] == d), fp8
        parts = np.arange(128, dtype=np.float32)
        ohT = (dstl[None, :] == parts[:, None]).astype(ml_dtypes.float8_e4m3)
        return {"ift": ift, "dstlT": dstlT, "ohT": ohT}


def make_waug(W, attn_l, attn_r):
    """[IN, 272] f32 cols: [W'^T | Ml | Mr]; W' rows in d-major order d*H+h
    (the DVE runs the edge-weight multiply fastest with the exp broadcast
    striding 1 over the 8 heads innermost — measured)."""
    perm = np.empty(HD, dtype=np.int64)
    for h in range(H):
        for d in range(D):
            perm[d * H + h] = h * D + d
    Wp = W[perm, :]                                   # [256, IN]
    Ml = np.zeros((IN, H), dtype=np.float32)
    Mr = np.zeros((IN, H), dtype=np.float32)
    for h in range(H):
        rows = W[h * D:(h + 1) * D, :]                # [D, IN]
        Ml[:, h] = attn_l[0, h, :] @ rows
        Mr[:, h] = attn_r[0, h, :] @ rows
    return np.concatenate([Wp.T, Ml, Mr], axis=1).astype(np.float32)


def make_consts():
    """[128, 2, 128] bf16: [iota_free | identity]."""
    iota = np.broadcast_to(np.arange(128, dtype=np.float32), (128, 128))
    ident = np.eye(128, dtype=np.float32)
    return np.stack([iota, ident], axis=1).astype(ml_dtypes.bfloat16)


def dma_gather_raw(gp, out_ap, in_ap, idxs_ap, num_idxs, elem_size, elem_step,
                   queue_num=0):
    """dma_gather minus the elem_size%256 assert (row stride must be %256B)."""
    stride_bytes = elem_step * mybir.dt.size(in_ap.dtype)
    stride_bytes_256 = exact_div(stride_bytes, 256)
    _in_ap = gp.lower_ap_dma(in_ap, for_custom_bir_dma=True)
    _idxs_ap = gp.lower_ap(idxs_ap)
    _out_ap = gp.lower_ap(out_ap)
    return gp.add_instruction(
        mybir.InstDMAGatherAnt(
            name=gp.bass.get_next_instruction_name(),
            ins=[*_in_ap, _idxs_ap, gp.lower_val_access(gp.to_reg(num_idxs))],
            outs=[_out_ap],
            transpose=False, num_idxs=num_idxs, elem_size=elem_size,
            stride_bytes_256=stride_bytes_256, gen_mode=0, single_packet=True,
            queue_num=queue_num, sbuf_tokens_per_rank=0, sbuf_free_dim_per_rank=0,
            sbuf_free_dim_pad_per_rank=0, sbuf_byte_offset=0,
        )
    )


def build_program(plan, n_cores, fc_mega=16, nq=3, ostg=4, ebufs=6):
    """One SPMD Bass program, A/B interleaved by chunk: FC for chunk q is
    emitted just before chunk q's edge windows, sharing one pool scope so
    engines pipeline across phases. Inputs: featT bf16 [IN,NN], waug bf16,
    ift i16, dstlT bf16, consts bf16. Output: out [128, NT*256] f32."""
    p = plan
    NN, ND, NT, WMAX = p.NN, p.ND, p.NT, p.WMAX
    nc = bacc.Bacc("TRN2", target_bir_lowering=False, debug=False,
                   num_devices=n_cores, num_swdge_queues=nq)

    featT_d = nc.dram_tensor("featT", [IN, NN], BF16, kind="ExternalInput").ap()
    waug_d = nc.dram_tensor("waug", [IN, HD + 16], BF16, kind="ExternalInput").ap()
    ift_d = nc.dram_tensor("ift", [128, p.NBtot // 16], I16,
                           kind="ExternalInput").ap()
    dstlT_d = nc.dram_tensor("dstlT", [128, p.NBtot // 128], BF16,
                             kind="ExternalInput").ap()
    ohT_d = nc.dram_tensor("ohT", [128, p.NBtot], FP8,
                           kind="ExternalInput").ap()
    consts_d = nc.dram_tensor("consts", [128, 2, 128], BF16,
                              kind="ExternalInput").ap()
    n_nt = NN // 128
    ft_ts = []
    rows_q = []
    for qq in range(p.NQ):
        rows = p.bounds[qq + 1] - p.bounds[qq]
        rows_q.append(rows)
        ft_ts.append(nc.dram_tensor(f"ft_tab{qq}", [rows, FT_W], BF16,
                                    kind="Internal").ap())
    eler_t = nc.dram_tensor("eler_tab", [128, NT * H], BF16, kind="Internal").ap()
    out_d = nc.dram_tensor("out", [128, NT * HD], F32, kind="ExternalOutput").ap()

    MG = fc_mega
    assert all(r % (MG * 128) == 0 for r in rows_q)

    # windows grouped by chunk
    wins_by_q = [[] for _ in range(p.NQ)]
    for w, (q, t, wn, pos) in enumerate(p.windows):
        wins_by_q[q].append((w, t, wn, pos))

    with tile.TileContext(nc) as tc:
        with tc.tile_pool(name="fca", bufs=2) as apool, \
             tc.tile_pool(name="fcc", bufs=1) as cpool, \
             tc.tile_pool(name="eb", bufs=ebufs) as pool, \
             tc.tile_pool(name="ebo", bufs=2) as opool, \
             tc.tile_pool(name="fcp", bufs=2, space="PSUM") as fpsp, \
             tc.tile_pool(name="ebp", bufs=2, space="PSUM") as psp, \
             tc.tile_pool(name="ebx", bufs=2, space="PSUM") as psx:
            wa = cpool.tile([128, 2, HD + 16], BF16)
            nc.sync.dma_start(wa[:], waug_d.rearrange("(k p) c -> p k c", p=128))
            consts = cpool.tile([128, 2, 128], BF16)
            nc.sync.dma_start(consts[:], consts_d)
            iotaF = consts[:, 0, :]
            identB = consts[:, 1, :]
            ift_all = cpool.tile([128, p.NBtot // 16], I16)
            nc.sync.dma_start(ift_all[:], ift_d)
            dstlT = cpool.tile([128, p.NBtot // 128], BF16)
            nc.sync.dma_start(dstlT[:], dstlT_d)
            acc = cpool.tile([128, NT, HD], BF16)      # cross-chunk numerator acc
            accd = cpool.tile([128, NT, H], F32)       # denominator acc (f32)
            er_all = cpool.tile([128, NT, H], BF16)

            agg = {}
            issued = {t: 0 for t in range(NT)}
            issued_q = {}
            ost = {}

            def fc_chunk(qq):
                g0q = p.bounds[qq] // 128
                gq = rows_q[qq] // 128          # rows per partition, p-major
                for lg in range(0, gq, MG):
                    yield
                    g0 = g0q + lg
                    gn = min(MG, n_nt - g0)
                    ftin = apool.tile([128, 2, MG * 128], BF16, tag="ftin")
                    nc.sync.dma_start(
                        ftin[:, :, :gn * 128],
                        featT_d.rearrange("(k p) n -> p k n", p=128)[:, :, g0 * 128:(g0 + gn) * 128])
                    ftst = apool.tile([128, MG, FT_W], BF16, tag="ftst")
                    elst = apool.tile([128, MG, H], BF16, tag="elst")
                    write_el = qq == 0 and g0 < NT
                    for j in range(gn):
                        fc_ps = fpsp.tile([128, HD + 16], F32, tag="fc")
                        for k in range(2):
                            nc.tensor.matmul(fc_ps[:], ftin[:, k, j * 128:(j + 1) * 128],
                                             wa[:, k, :], start=(k == 0), stop=(k == 1))
                        if j % 2 == 0:
                            nc.vector.tensor_copy(ftst[:, j, 0:ROW], fc_ps[:, 0:ROW])
                        else:
                            nc.scalar.copy(ftst[:, j, 0:ROW], fc_ps[:, 0:ROW])
                        if write_el and g0 + j < NT:
                            nc.scalar.copy(elst[:, j, :], fc_ps[:, HD + 8:HD + 16])
                    # p-major rows: partition p holds rows p*gq + lg .. +gn.
                    # Full 384-col rows (pad cols carry stale SBUF junk) so the
                    # HWDGE write is one contiguous 12KB run per partition.
                    nc.sync.dma_start(
                        ft_ts[qq].rearrange("(p g) c -> p g c", p=128)[:, lg:lg + gn, :],
                        ftst[:, :gn, :])
                    if write_el:
                        en = min(gn, NT - g0)
                        nc.scalar.dma_start(eler_t[:, g0 * H:(g0 + en) * H],
                                            elst[:, :en, :])

            gens = [fc_chunk(qq) for qq in range(p.NQ)]
            for _ in gens[0]:
                pass
            nc.scalar.dma_start(
                er_all[:],
                eler_t[:].rearrange("p (g c) -> p g c", c=H))
            for q in range(p.NQ):
                nxt = gens[q + 1] if q + 1 < p.NQ else None
                wq = wins_by_q[q]
                ngroups = ((p.bounds[q + 2] - p.bounds[q + 1]) // 128
                           + MG - 1) // MG if nxt is not None else 0
                gi = 0
                for wi, (wcount, t, wn, pos) in enumerate(wq):
                    NB = wn * 128

                    g = pool.tile([128, WMAX, ROW], BF16, tag="g")
                    dma_gather_raw(nc.gpsimd, g[:, :wn, :],
                                   ft_ts[q][:, 0:ROW],
                                   ift_all[:, pos // 16:(pos + NB) // 16],
                                   NB, ROW, FT_W,
                                   queue_num=wcount % nq)

                    # one-hots on-chip: ohs[p=edge, j, d] = (dstl[p] == d)
                    b0 = pos // 128
                    ohs = pool.tile([128, WMAX, 128], BF16, tag="ohs")
                    nc.vector.tensor_tensor(
                        ohs[:, :wn, :],
                        dstlT[:, b0:b0 + wn].unsqueeze(2)
                            .broadcast_to([128, wn, 128]),
                        iotaF.unsqueeze(1).broadcast_to([128, wn, 128]),
                        mybir.AluOpType.is_equal)
                    # ohT[d, j, i] = (dstl[i] == d): streamed fp8 from host
                    ohT = pool.tile([128, WMAX, 128], FP8, tag="ohT")
                    nc.scalar.dma_start(
                        ohT[:, :wn, :].rearrange("p b i -> p (b i)"),
                        ohT_d[:, pos:pos + NB])

                    # er per edge: erx[i, j, h] = er_all[dstl[i], t, h]
                    erx_ps = psx.tile([128, WMAX, H], F32, tag="erx",
                                      name=f"erx{wcount}")
                    for j in range(wn):
                        nc.tensor.matmul(erx_ps[:, j, :], ohT[:, j, :],
                                         er_all[:, t, :], start=True, stop=True,
                                         skip_group_check=True)
                    lw = pool.tile([128, WMAX, H], F32, tag="lw")
                    nc.vector.tensor_tensor(
                        lw[:, :wn, :], g[:, :wn, HD:HD + 8], erx_ps[:, :wn, :],
                        mybir.AluOpType.add)
                    nc.vector.scalar_tensor_tensor(lw[:, :wn, :], lw[:, :wn, :],
                                                   ALPHA, lw[:, :wn, :],
                                                   mybir.AluOpType.mult,
                                                   mybir.AluOpType.max)
                    rhs = pool.tile([128, WMAX, HD + 8], BF16, tag="rhs")
                    nc.scalar.activation(rhs[:, :wn, HD:HD + 8], lw[:, :wn, :],
                                         mybir.ActivationFunctionType.Exp)
                    nc.vector.tensor_tensor(
                        rhs[:, :wn, 0:HD].rearrange("p b (d h) -> p b d h", h=H),
                        g[:, :wn, 0:HD].rearrange("p b (d h) -> p b d h", h=H),
                        rhs[:, :wn, HD:HD + 8].unsqueeze(2)
                            .broadcast_to([128, wn, D, H]),
                        mybir.AluOpType.mult)

                    cs = p.nb[(q, t)]
                    if (q, t) not in issued_q:
                        agg[t] = psp.tile([128, HD + 8], F32, tag="agg",
                                          name=f"agg{q}_{t}")
                        issued_q[(q, t)] = 0
                    at = agg[t]
                    for j in range(wn):
                        nc.tensor.matmul(at[:], ohs[:, j, :], rhs[:, j, :],
                                         start=(issued_q[(q, t)] == 0),
                                         stop=(issued_q[(q, t)] == cs - 1),
                                         skip_group_check=True)
                        issued_q[(q, t)] += 1
                        issued[t] += 1
                    if issued_q[(q, t)] == cs:
                        # fold chunk-partial into SBUF accumulators
                        if q == 0:
                            nc.vector.tensor_copy(acc[:, t, :], at[:, 0:HD])
                            nc.vector.tensor_scalar(accd[:, t, :],
                                                    at[:, HD:HD + 8],
                                                    1e-20, None,
                                                    mybir.AluOpType.add)
                        else:
                            nc.vector.tensor_tensor(
                                acc[:, t, :], acc[:, t, :], at[:, 0:HD],
                                mybir.AluOpType.add)
                            nc.vector.tensor_tensor(
                                accd[:, t, :], accd[:, t, :], at[:, HD:HD + 8],
                                mybir.AluOpType.add)
                        del agg[t]
                        del issued_q[(q, t)]
                    if issued[t] == p.tot_segs[t]:
                        s = t // ostg
                        if s not in ost:
                            ost[s] = opool.tile([128, ostg, HD], F32,
                                                tag="ost", name=f"ost{s}")
                        pool_ost = ost[s]
                        recd = pool.tile([128, H], F32, tag="recd")
                        nc.vector.reciprocal(recd[:], accd[:, t, :])
                        nc.vector.tensor_tensor(
                            pool_ost[:, t % ostg, :].rearrange(
                                "p (h d) -> p h d", d=D),
                            acc[:, t, :].rearrange("p (d h) -> p h d", h=H),
                            recd[:].unsqueeze(2).broadcast_to([128, H, D]),
                            mybir.AluOpType.mult)
                        t0 = s * ostg
                        n_in_st = min(ostg, NT - t0)
                        if all(issued[tt] == p.tot_segs[tt]
                               for tt in range(t0, t0 + n_in_st)):
                            nc.scalar.dma_start(
                                out_d[:, t0 * HD:(t0 + n_in_st) * HD]
                                .rearrange("p (g c) -> p g c", c=HD),
                                pool_ost[:, :n_in_st, :])
                            del ost[s]
                    # spread next chunk's FC groups over the first ~60% of
                    # this chunk's windows so its table is ready at the
                    # chunk boundary (in-order engine queues).
                    if nxt is not None and wq:
                        want = min(ngroups, ((wi + 1) * ngroups * 5) // (len(wq) * 3))
                        while gi < want and next(nxt, 0) is None:
                            gi += 1
                if nxt is not None:
                    while next(nxt, 0) is None:
                        gi += 1
    return _finish(nc)


def _finish(nc):
    nc.compile()
    return nc


def host_prep(feat, W, attn_l, attn_r, src, dst, n_cores, tiles_per_core,
              chunk=32768, wmax=8):
    N = feat.shape[0]
    E = src.shape[0]
    plan = Plan(N, E, src.astype(np.int64), dst.astype(np.int64), n_cores,
                tiles_per_core, chunk, wmax)
    featb = np.zeros((plan.ND * n_cores, IN), dtype=ml_dtypes.bfloat16)
    featb[:N] = feat.astype(ml_dtypes.bfloat16)
    waug = make_waug(W, attn_l, attn_r).astype(ml_dtypes.bfloat16)
    consts = make_consts()
    in_maps = []
    for c in range(n_cores):
        s = plan.build_streams(c)
        ftc = np.zeros((IN, plan.NN), dtype=ml_dtypes.bfloat16)
        oc = plan.node_order[c]
        ftc[:, :len(oc)] = featb[oc].T
        in_maps.append({
            "featT": ftc, "waug": waug, "consts": consts,
            "ift": s["ift"], "dstlT": s["dstlT"], "ohT": s["ohT"],
        })
    return plan, in_maps


def assemble_output(plan, results, N):
    full = np.zeros((plan.ND * plan.C, HD), dtype=np.float32)
    for c in range(plan.C):
        r = results[c]["out"].reshape(128, plan.NT, HD)
        full[c * plan.ND:(c + 1) * plan.ND] = (
            r.transpose(1, 0, 2).reshape(plan.ND, HD))
    return full[:N].reshape(N, H, D)


# ----------------------------------------------------------------------------
# Harness entrypoint: full inputs in, full output out. Shapes hardcoded for
# nn_GATConv (N=100000, E=1600000, IN=256, H=8, D=32) on 8 NeuronCores.
# ----------------------------------------------------------------------------
from concourse.bass_interp import get_hw_module as _get_hw_module
from concourse import bass_utils as _bass_utils

_N_CORES = 8
_TPC = 98            # dst tiles per core (98*128*8 = 100352 >= 100000)
_CHUNK = 32768
_WMAX = 8
_NQ = 4              # SWDGE queues: windows alternate queues
_OSTG = 4
_EBUFS = 6

_cache = {}


def kernel(feat, W, attn_l, attn_r, src, dst):
    feat = np.ascontiguousarray(np.asarray(feat, dtype=np.float32))
    W = np.ascontiguousarray(np.asarray(W, dtype=np.float32))
    attn_l = np.asarray(attn_l, dtype=np.float32)
    attn_r = np.asarray(attn_r, dtype=np.float32)
    src = np.asarray(src).astype(np.int64)
    dst = np.asarray(dst).astype(np.int64)
    N = feat.shape[0]

    plan, in_maps = host_prep(feat, W, attn_l, attn_r, src, dst,
                              _N_CORES, _TPC, chunk=_CHUNK, wmax=_WMAX)
    key = "prog"
    if key not in _cache:
        nc = build_program(plan, _N_CORES, nq=_NQ, ostg=_OSTG, ebufs=_EBUFS)
        nc.m = _get_hw_module(nc.m)
        _cache[key] = nc
    nc = _cache[key]
    res = _bass_utils.run_bass_kernel_spmd(nc, in_maps,
                                           core_ids=list(range(_N_CORES)))
    return assemble_output(plan, res.results, N)
